# revision 3
# baseline (speedup 1.0000x reference)
"""DGIN (D-MPNN + GIN) message-passing network on 8 Trainium2 NeuronCores.

Strategy (row-sharded 1D graph parallel, per the sharding hint):
  - Edge rows (E=8192) and node rows (N=4096) are split 8 ways; each core owns
    the matching row slices of A_e [E,E], A_ne [N,E], A [N,N] and feature rows.
  - Adjacency matrices are streamed in fp32 once, transposed on the TensorE
    (PE) 128x128 at a time, and cached in SBUF as bf16 A^T tiles (k on
    partitions) so every dense adjacency matmul can contract over the
    partition axis.  A_e^T (16 MB bf16) stays resident in SBUF for all 3
    D-MPNN iterations - the f32 matrix is read from HBM exactly once.
  - The state matrix h (small: [E,64]/[N,128] bf16) is replicated; after each
    per-shard update it is AllGathered (bf16, HBM bounce) so the next
    iteration's contraction sees all rows.
  - LayerNorm is per-row and runs in the natural row-on-partition layout
    (free-axis reductions on VectorE); matmul outputs that appear transposed
    are turned back with PE transposes.
  - identity_matrix input is mathematically unused by the reference and is
    never shipped to the device.
"""

import contextlib

import numpy as np

import concourse.bass as bass
import concourse.mybir as mybir
import concourse.tile as tile
from concourse import bacc
from concourse.bass_utils import run_bass_kernel_spmd
from concourse.masks import make_identity

dt = mybir.dt
Alu = mybir.AluOpType
Act = mybir.ActivationFunctionType

# problem shape (hardcoded per contest contract)
E, N = 8192, 4096
NF, EFD = 64, 16
H = 64
G = NF + H  # 128
TD, TG = 3, 3
EPS_LN = 1e-6

P = 128
C = 8
ES = E // C           # 1024 edge rows per core
NS = N // C           # 512 node rows per core
KBE = E // P          # 64 contraction blocks over E
KBN = N // P          # 32 contraction blocks over N
ET = ES // P          # 8 edge row-tiles per shard
NT = NS // P          # 4 node row-tiles per shard


def _build():
    nc = bacc.Bacc("TRN2", target_bir_lowering=False, debug=False,
                   enable_asserts=True, num_devices=C)

    # ---- per-core external inputs ----
    ae_s = nc.dram_tensor("ae_s", [ES, E], dt.float32, kind="ExternalInput")
    ane_s = nc.dram_tensor("ane_s", [NS, E], dt.float32, kind="ExternalInput")
    a_s = nc.dram_tensor("a_s", [NS, N], dt.float32, kind="ExternalInput")
    eal_s = nc.dram_tensor("eal_s", [ES, NF], dt.float32, kind="ExternalInput")
    ef_s = nc.dram_tensor("ef_s", [ES, EFD], dt.float32, kind="ExternalInput")
    nf_s = nc.dram_tensor("nf_s", [NS, NF], dt.float32, kind="ExternalInput")
    w_init = nc.dram_tensor("w_init", [NF + EFD, H], dt.float32, kind="ExternalInput")
    b_init = nc.dram_tensor("b_init", [H, 1], dt.float32, kind="ExternalInput")
    w_pass = nc.dram_tensor("w_pass", [TD, H, H], dt.float32, kind="ExternalInput")
    w_gin = nc.dram_tensor("w_gin", [TG, G, G], dt.float32, kind="ExternalInput")
    b_gin = nc.dram_tensor("b_gin", [TG, G], dt.float32, kind="ExternalInput")
    eps_t = nc.dram_tensor("eps_t", [1, G], dt.float32, kind="ExternalInput")
    g1_t = nc.dram_tensor("g1_t", [1, H], dt.float32, kind="ExternalInput")
    b1_t = nc.dram_tensor("b1_t", [1, H], dt.float32, kind="ExternalInput")
    g2_t = nc.dram_tensor("g2_t", [1, H], dt.float32, kind="ExternalInput")
    b2_t = nc.dram_tensor("b2_t", [1, H], dt.float32, kind="ExternalInput")
    g3_t = nc.dram_tensor("g3_t", [1, G], dt.float32, kind="ExternalInput")
    b3_t = nc.dram_tensor("b3_t", [1, G], dt.float32, kind="ExternalInput")
    g4_t = nc.dram_tensor("g4_t", [1, G], dt.float32, kind="ExternalInput")
    b4_t = nc.dram_tensor("b4_t", [1, G], dt.float32, kind="ExternalInput")

    out = nc.dram_tensor("out", [1, G], dt.float32, kind="ExternalOutput")

    # ---- internal DRAM bounce buffers for collectives ----
    agh_in = [nc.dram_tensor(f"agh_in{k}", [ES, H], dt.bfloat16) for k in range(4)]
    agh_out = [nc.dram_tensor(f"agh_out{k}", [E, H], dt.bfloat16) for k in range(4)]
    agv_in = [nc.dram_tensor(f"agv_in{k}", [NS, G], dt.bfloat16) for k in range(3)]
    agv_out = [nc.dram_tensor(f"agv_out{k}", [N, G], dt.bfloat16) for k in range(3)]
    ar_in = nc.dram_tensor("ar_in", [1, G], dt.float32)
    ar_out = nc.dram_tensor("ar_out", [1, G], dt.float32)

    rg = [list(range(C))]
    drain_cnt = [0]

    def drain(out_ap, in_ap):
        """PSUM -> SBUF copy, alternating DVE / ACT to split the load."""
        drain_cnt[0] += 1
        if drain_cnt[0] % 2 == 0:
            nc.vector.tensor_copy(out_ap, in_ap)
        else:
            nc.scalar.copy(out_ap, in_ap)

    with tile.TileContext(nc) as tc:
        stack = contextlib.ExitStack()
        const = stack.enter_context(tc.tile_pool(name="const", bufs=1))
        stream = stack.enter_context(tc.tile_pool(name="stream", bufs=3))
        psA = stack.enter_context(tc.tile_pool(name="psA", bufs=2, space="PSUM"))
        psT = stack.enter_context(tc.tile_pool(name="psT", bufs=3, space="PSUM"))
        psW = stack.enter_context(tc.tile_pool(name="psW", bufs=2, space="PSUM"))

        # ================= P0: constants / parameters =================
        ident_f = const.tile([P, P], dt.float32)
        make_identity(nc, ident_f[:])
        ident_b = const.tile([P, P], dt.bfloat16)
        make_identity(nc, ident_b[:])
        ones_g = const.tile([P, 1], dt.bfloat16)
        nc.vector.memset(ones_g[:], 1.0)
        epsb = const.tile([P, 1], dt.float32)
        nc.vector.memset(epsb[:], EPS_LN)

        w_init_sb = const.tile([P, H], dt.bfloat16)
        nc.vector.memset(w_init_sb[:], 0.0)
        nc.gpsimd.dma_start(w_init_sb[: NF + EFD, :], w_init[:])  # cast f32->bf16
        b_init_sb = const.tile([H, 1], dt.float32)
        nc.sync.dma_start(b_init_sb[:], b_init[:])

        w_pass_sb = const.tile([P, TD, H], dt.bfloat16)
        nc.vector.memset(w_pass_sb[:], 0.0)
        nc.gpsimd.dma_start(w_pass_sb[:H, :, :], w_pass[:].rearrange("t a b -> a t b"))
        w_gin_sb = const.tile([P, TG, G], dt.bfloat16)
        nc.gpsimd.dma_start(w_gin_sb[:], w_gin[:].rearrange("t a b -> a t b"))

        # LayerNorm gamma/beta, broadcast to all 128 partitions once
        def bcast_param(src, width):
            row = const.tile([1, width], dt.float32, name=f"row_{src.name}")
            nc.sync.dma_start(row[:], src[:])
            full = const.tile([P, width], dt.float32, name=f"bc_{src.name}")
            nc.gpsimd.partition_broadcast(full[:], row[:])
            return full

        g1m = bcast_param(g1_t, H)
        b1m = bcast_param(b1_t, H)
        g2m = bcast_param(g2_t, H)
        b2m = bcast_param(b2_t, H)
        g3m = bcast_param(g3_t, G)
        b3m = bcast_param(b3_t, G)
        epsm = bcast_param(eps_t, G)
        nc.vector.tensor_scalar_add(epsm[:], epsm[:], 1.0)  # (1 + eps)
        bginm = []
        for t in range(TG):
            row = const.tile([1, G], dt.float32, name=f"bgrow{t}")
            nc.sync.dma_start(row[:], b_gin[t:t + 1, :])
            full = const.tile([P, G], dt.float32, name=f"bginm{t}")
            nc.gpsimd.partition_broadcast(full[:], row[:])
            bginm.append(full)
        g4_sb = const.tile([1, G], dt.float32)
        nc.sync.dma_start(g4_sb[:], g4_t[:])
        b4_sb = const.tile([1, G], dt.float32)
        nc.sync.dma_start(b4_sb[:], b4_t[:])

        # D-MPNN-scoped pool (P1..P4)
        dm = stack.enter_context(tc.tile_pool(name="dm", bufs=1))

        # =============== P1: h0 = relu([eal|ef] @ W_init + b) ===============
        xt = dm.tile([P, ES], dt.bfloat16)  # X^T, zero-padded rows 80:128
        nc.vector.memset(xt[:], 0.0)
        for et in range(ET):
            eal_tile = stream.tile([P, NF], dt.float32, tag="feat")
            nc.sync.dma_start(eal_tile[:], eal_s[et * P:(et + 1) * P, :])
            ps = psT.tile([NF, P], dt.float32, tag="pst")
            nc.tensor.transpose(ps[:], eal_tile[:], ident_f[:])
            drain(xt[:NF, et * P:(et + 1) * P], ps[:])
            ef_tile = stream.tile([P, EFD], dt.float32, tag="feat2")
            nc.sync.dma_start(ef_tile[:], ef_s[et * P:(et + 1) * P, :])
            ps2 = psT.tile([EFD, P], dt.float32, tag="pst")
            nc.tensor.transpose(ps2[:], ef_tile[:], ident_f[:])
            drain(xt[NF:NF + EFD, et * P:(et + 1) * P], ps2[:])

        h0T = dm.tile([H, ES], dt.bfloat16)
        for nh in range(ES // 512):
            psh = psA.tile([H, 512], dt.float32, tag="acc", name=f"h0ps{nh}")
            nc.tensor.matmul(psh[:], w_init_sb[:], xt[:, nh * 512:(nh + 1) * 512],
                             start=True, stop=True)
            nc.scalar.activation(h0T[:, nh * 512:(nh + 1) * 512], psh[:],
                                 Act.Relu, bias=b_init_sb[:])

        h0n = dm.tile([P, ET, H], dt.bfloat16)  # h0 natural, kept all D-MPNN
        for et in range(ET):
            ps = psT.tile([P, H], dt.bfloat16, tag="pst", name=f"h0n{et}")
            nc.tensor.transpose(ps[:], h0T[:, et * P:(et + 1) * P], ident_b[:H, :H])
            drain(h0n[:, et, :], ps[:])

        nc.sync.dma_start(agh_in[0][:].rearrange("(et p) h -> p et h", p=P), h0n[:])
        nc.gpsimd.collective_compute("AllGather", Alu.bypass, replica_groups=rg,
                                     ins=[agh_in[0][:]], outs=[agh_out[0][:]])

        # D-MPNN working tiles (shared with P4, which uses sub-ranges)
        mT = dm.tile([H, ES], dt.bfloat16)
        m_nat = dm.tile([P, ET, H], dt.bfloat16)
        mu = dm.tile([P, ET], dt.float32)
        var = dm.tile([P, ET], dt.float32)
        rstd = dm.tile([P, ET], dt.float32)
        cen = dm.tile([P, ET, H], dt.float32)
        t1 = dm.tile([P, ET, H], dt.float32)
        lnT = dm.tile([P, ET, P], dt.bfloat16)
        nc.vector.memset(lnT[:], 0.0)  # zero rows 64:128 once (K=128 pad)

        h_cur = dm.tile([P, KBE, H], dt.bfloat16, tag="hsb", bufs=2, name="h_cur0")
        nc.sync.dma_start(h_cur[:], agh_out[0][:].rearrange("(kb p) h -> p kb h", p=P))

        # =============== P2: build A_e^T bf16 cache in SBUF ===============
        with tc.tile_pool(name="atc_pool", bufs=1) as atc_pool:
            atc = atc_pool.tile([P, KBE, ES], dt.bfloat16)  # 128 KiB/partition
            for et in range(ET):
                for kc in range(4):
                    chunk = stream.tile([P, 2048], dt.float32, tag="achunk", bufs=2)
                    nc.sync.dma_start(
                        chunk[:], ae_s[et * P:(et + 1) * P, kc * 2048:(kc + 1) * 2048])
                    for j in range(16):
                        kb = kc * 16 + j
                        ps = psT.tile([P, P], dt.float32, tag="pst")
                        nc.tensor.transpose(ps[:], chunk[:, j * P:(j + 1) * P], ident_f[:])
                        drain(atc[:, kb, et * P:(et + 1) * P], ps[:])

            # =============== P3: D-MPNN iterations ===============
            for t in range(TD):
                # --- m^T = (A_e[rows] @ h)^T via lhsT=h blocks, rhs=A^T ---
                for nh in range(ES // 512):
                    ps = psA.tile([H, 512], dt.float32, tag="acc", name=f"mT{t}_{nh}")
                    for kb in range(KBE):
                        nc.tensor.matmul(ps[:], h_cur[:, kb, :],
                                         atc[:, kb, nh * 512:(nh + 1) * 512],
                                         start=(kb == 0), stop=(kb == KBE - 1))
                    drain(mT[:, nh * 512:(nh + 1) * 512], ps[:])
                # --- back to natural layout ---
                for et in range(ET):
                    ps = psT.tile([P, H], dt.bfloat16, tag="pst", name=f"mn{t}_{et}")
                    nc.tensor.transpose(ps[:], mT[:, et * P:(et + 1) * P],
                                        ident_b[:H, :H])
                    drain(m_nat[:, et, :], ps[:])
                # --- LayerNorm (g1, b1) over H, rows on partitions ---
                nc.vector.tensor_reduce(mu[:], m_nat[:], mybir.AxisListType.X, Alu.add)
                nc.scalar.mul(mu[:], mu[:], 1.0 / H)
                for et in range(ET):
                    nc.vector.tensor_scalar_sub(cen[:, et, :], m_nat[:, et, :],
                                                mu[:, et:et + 1])
                    nc.scalar.activation(t1[:, et, :], cen[:, et, :], Act.Square,
                                         accum_out=var[:, et:et + 1])
                nc.scalar.activation(rstd[:], var[:], Act.Sqrt, scale=1.0 / H,
                                     bias=epsb[:])
                nc.vector.reciprocal(rstd[:], rstd[:])
                ln_nat = stream.tile([P, ET, H], dt.bfloat16, tag="lnn",
                                     name=f"ln{t}")
                for et in range(ET):
                    nc.vector.scalar_tensor_tensor(t1[:, et, :], cen[:, et, :],
                                                   rstd[:, et:et + 1], g1m[:],
                                                   Alu.mult, Alu.mult)
                    nc.vector.tensor_add(ln_nat[:, et, :], t1[:, et, :], b1m[:])
                # --- transpose ln for the W_pass matmul ---
                for et in range(ET):
                    ps = psT.tile([H, P], dt.bfloat16, tag="pst", name=f"lt{t}_{et}")
                    nc.tensor.transpose(ps[:], ln_nat[:, et, :], ident_b[:])
                    drain(lnT[:H, et, :], ps[:])
                # --- h = relu(h0 + ln @ W_pass[t]) ---
                h_new = dm.tile([P, ET, H], dt.bfloat16, tag="hnew", bufs=2,
                                name=f"h_new{t}")
                for et in range(ET):
                    psw = psW.tile([P, H], dt.float32, tag="wps", name=f"wp{t}_{et}")
                    nc.tensor.matmul(psw[:], lnT[:, et, :], w_pass_sb[:, t, :],
                                     start=True, stop=True)
                    nc.vector.tensor_add(t1[:, et, :], psw[:], h0n[:, et, :])
                    nc.scalar.activation(h_new[:, et, :], t1[:, et, :], Act.Relu)
                # --- AllGather updated h ---
                nc.sync.dma_start(
                    agh_in[t + 1][:].rearrange("(et p) h -> p et h", p=P), h_new[:])
                nc.gpsimd.collective_compute(
                    "AllGather", Alu.bypass, replica_groups=rg,
                    ins=[agh_in[t + 1][:]], outs=[agh_out[t + 1][:]])
                h_cur = dm.tile([P, KBE, H], dt.bfloat16, tag="hsb", bufs=2,
                                name=f"h_cur{t + 1}")
                nc.sync.dma_start(
                    h_cur[:], agh_out[t + 1][:].rearrange("(kb p) h -> p kb h", p=P))

        # =============== P4: m_v = LN(A_ne[rows] @ h); h0_v = [nf | m_v] =======
        hv = stack.enter_context(tc.tile_pool(name="hv", bufs=1))
        h0v = hv.tile([P, NT, G], dt.bfloat16)
        nc.gpsimd.dma_start(h0v[:, :, :NF],
                            nf_s[:].rearrange("(nt p) f -> p nt f", p=P))
        with tc.tile_pool(name="atne_pool", bufs=1) as atne_pool:
            atne = atne_pool.tile([P, KBE, NS], dt.bfloat16)  # 64 KiB/partition
            for rt in range(NT):
                for kc in range(4):
                    chunk = stream.tile([P, 2048], dt.float32, tag="achunk", bufs=2)
                    nc.sync.dma_start(
                        chunk[:], ane_s[rt * P:(rt + 1) * P, kc * 2048:(kc + 1) * 2048])
                    for j in range(16):
                        eb = kc * 16 + j
                        ps = psT.tile([P, P], dt.float32, tag="pst")
                        nc.tensor.transpose(ps[:], chunk[:, j * P:(j + 1) * P],
                                            ident_f[:])
                        drain(atne[:, eb, rt * P:(rt + 1) * P], ps[:])

            psv = psA.tile([H, NS], dt.float32, tag="acc", name="mvT")
            for eb in range(KBE):
                nc.tensor.matmul(psv[:], h_cur[:, eb, :], atne[:, eb, :],
                                 start=(eb == 0), stop=(eb == KBE - 1))
            drain(mT[:, :NS], psv[:])

        for nt in range(NT):
            ps = psT.tile([P, H], dt.bfloat16, tag="pst", name=f"mv{nt}")
            nc.tensor.transpose(ps[:], mT[:, nt * P:(nt + 1) * P], ident_b[:H, :H])
            drain(m_nat[:, nt, :], ps[:])
        # LayerNorm (g2, b2) into h0v[:, :, 64:128]
        nc.vector.tensor_reduce(mu[:, :NT], m_nat[:, :NT, :], mybir.AxisListType.X,
                                Alu.add)
        nc.scalar.mul(mu[:, :NT], mu[:, :NT], 1.0 / H)
        for nt in range(NT):
            nc.vector.tensor_scalar_sub(cen[:, nt, :], m_nat[:, nt, :],
                                        mu[:, nt:nt + 1])
            nc.scalar.activation(t1[:, nt, :], cen[:, nt, :], Act.Square,
                                 accum_out=var[:, nt:nt + 1])
        nc.scalar.activation(rstd[:, :NT], var[:, :NT], Act.Sqrt, scale=1.0 / H,
                             bias=epsb[:])
        nc.vector.reciprocal(rstd[:, :NT], rstd[:, :NT])
        for nt in range(NT):
            nc.vector.scalar_tensor_tensor(t1[:, nt, :], cen[:, nt, :],
                                           rstd[:, nt:nt + 1], g2m[:],
                                           Alu.mult, Alu.mult)
            nc.vector.tensor_add(h0v[:, nt, NF:G], t1[:, nt, :], b2m[:])

        nc.sync.dma_start(agv_in[0][:].rearrange("(nt p) g -> p nt g", p=P), h0v[:])
        nc.gpsimd.collective_compute("AllGather", Alu.bypass, replica_groups=rg,
                                     ins=[agv_in[0][:]], outs=[agv_out[0][:]])

        # dm pool (A_e-phase working set) no longer needed
        # (it stays open in the stack; its big tiles are last-used above)

        # =============== P5/P6: GIN iterations ===============
        with tc.tile_pool(name="atc2_pool", bufs=1) as atc2_pool:
            atc2 = atc2_pool.tile([P, KBN, NS], dt.bfloat16)  # 32 KiB/partition
            for rt in range(NT):
                for kc in range(2):
                    chunk = stream.tile([P, 2048], dt.float32, tag="achunk", bufs=2)
                    nc.sync.dma_start(
                        chunk[:], a_s[rt * P:(rt + 1) * P, kc * 2048:(kc + 1) * 2048])
                    for j in range(16):
                        kb = kc * 16 + j
                        ps = psT.tile([P, P], dt.float32, tag="pst")
                        nc.tensor.transpose(ps[:], chunk[:, j * P:(j + 1) * P],
                                            ident_f[:])
                        drain(atc2[:, kb, rt * P:(rt + 1) * P], ps[:])

            hv_cur = hv.tile([P, KBN, G], dt.bfloat16, tag="hvsb", bufs=2, name="hv0")
            nc.sync.dma_start(hv_cur[:],
                              agv_out[0][:].rearrange("(kb p) g -> p kb g", p=P))

            zT = hv.tile([G, NS], dt.bfloat16)
            z_nat = hv.tile([P, NT, G], dt.bfloat16)
            muz = hv.tile([P, NT], dt.float32)
            varz = hv.tile([P, NT], dt.float32)
            rstdz = hv.tile([P, NT], dt.float32)
            cenz = hv.tile([P, NT, G], dt.float32)
            t1z = hv.tile([P, NT, G], dt.float32)
            hv_final = None
            for t in range(TG):
                psz = psA.tile([G, NS], dt.float32, tag="acc", name=f"zT{t}")
                for kb in range(KBN):
                    nc.tensor.matmul(psz[:], hv_cur[:, kb, :], atc2[:, kb, :],
                                     start=(kb == 0), stop=(kb == KBN - 1))
                drain(zT[:], psz[:])
                for nt in range(NT):
                    ps = psT.tile([P, G], dt.bfloat16, tag="pst", name=f"zn{t}_{nt}")
                    nc.tensor.transpose(ps[:], zT[:, nt * P:(nt + 1) * P], ident_b[:])
                    drain(z_nat[:, nt, :], ps[:])
                # LayerNorm (g3, b3) over G
                nc.vector.tensor_reduce(muz[:], z_nat[:], mybir.AxisListType.X,
                                        Alu.add)
                nc.scalar.mul(muz[:], muz[:], 1.0 / G)
                for nt in range(NT):
                    nc.vector.tensor_scalar_sub(cenz[:, nt, :], z_nat[:, nt, :],
                                                muz[:, nt:nt + 1])
                    nc.scalar.activation(t1z[:, nt, :], cenz[:, nt, :], Act.Square,
                                         accum_out=varz[:, nt:nt + 1])
                nc.scalar.activation(rstdz[:], varz[:], Act.Sqrt, scale=1.0 / G,
                                     bias=epsb[:])
                nc.vector.reciprocal(rstdz[:], rstdz[:])
                pre = stream.tile([P, NT, G], dt.bfloat16, tag="pre", name=f"pre{t}")
                for nt in range(NT):
                    nc.vector.scalar_tensor_tensor(t1z[:, nt, :], cenz[:, nt, :],
                                                   rstdz[:, nt:nt + 1], g3m[:],
                                                   Alu.mult, Alu.mult)
                    nc.vector.tensor_add(t1z[:, nt, :], t1z[:, nt, :], b3m[:])
                    # pre = (1+eps)*h0_v + LN(z)
                    nc.vector.tensor_mul(cenz[:, nt, :], h0v[:, nt, :], epsm[:])
                    nc.vector.tensor_add(pre[:, nt, :], cenz[:, nt, :], t1z[:, nt, :])
                # h_v = pre @ W_gin[t] + b_gin[t]
                preT = stream.tile([P, NT, P], dt.bfloat16, tag="preT",
                                   name=f"preT{t}")
                for nt in range(NT):
                    ps = psT.tile([P, P], dt.bfloat16, tag="pst", name=f"pT{t}_{nt}")
                    nc.tensor.transpose(ps[:], pre[:, nt, :], ident_b[:])
                    drain(preT[:, nt, :], ps[:])
                hv_new = hv.tile([P, NT, G], dt.bfloat16, tag="hvnew", bufs=2,
                                 name=f"hv_new{t}")
                for nt in range(NT):
                    psw = psW.tile([P, G], dt.float32, tag="wps", name=f"wg{t}_{nt}")
                    nc.tensor.matmul(psw[:], preT[:, nt, :], w_gin_sb[:, t, :],
                                     start=True, stop=True)
                    nc.vector.tensor_add(hv_new[:, nt, :], psw[:], bginm[t][:])
                if t < TG - 1:
                    nc.sync.dma_start(
                        agv_in[t + 1][:].rearrange("(nt p) g -> p nt g", p=P),
                        hv_new[:])
                    nc.gpsimd.collective_compute(
                        "AllGather", Alu.bypass, replica_groups=rg,
                        ins=[agv_in[t + 1][:]], outs=[agv_out[t + 1][:]])
                    hv_cur = hv.tile([P, KBN, G], dt.bfloat16, tag="hvsb", bufs=2,
                                     name=f"hv{t + 1}")
                    nc.sync.dma_start(
                        hv_cur[:],
                        agv_out[t + 1][:].rearrange("(kb p) g -> p kb g", p=P))
                else:
                    hv_final = hv_new

            # =============== P7: readout + final LayerNorm ===============
            ps_sum = psW.tile([1, G], dt.float32, tag="wps", name="ps_sum")
            for nt in range(NT):
                nc.tensor.matmul(ps_sum[:], ones_g[:], hv_final[:, nt, :],
                                 start=(nt == 0), stop=(nt == NT - 1))
            sum_sb = hv.tile([1, G], dt.float32)
            nc.vector.tensor_copy(sum_sb[:], ps_sum[:])
            nc.sync.dma_start(ar_in[:], sum_sb[:])
            nc.gpsimd.collective_compute("AllReduce", Alu.add, replica_groups=rg,
                                         ins=[ar_in[:]], outs=[ar_out[:]])
            gsum = hv.tile([1, G], dt.float32)
            nc.sync.dma_start(gsum[:], ar_out[:])

            s1 = hv.tile([1, 1], dt.float32)
            nc.vector.tensor_reduce(s1[:], gsum[:], mybir.AxisListType.X, Alu.add)
            nc.scalar.mul(s1[:], s1[:], 1.0 / G)
            cenf = hv.tile([1, G], dt.float32)
            nc.vector.tensor_scalar_sub(cenf[:], gsum[:], s1[:])
            sqf = hv.tile([1, G], dt.float32)
            varf = hv.tile([1, 1], dt.float32)
            nc.scalar.activation(sqf[:], cenf[:], Act.Square, accum_out=varf[:])
            nc.scalar.activation(varf[:], varf[:], Act.Sqrt, scale=1.0 / G,
                                 bias=epsb[:1, :])
            nc.vector.reciprocal(varf[:], varf[:])
            outf = hv.tile([1, G], dt.float32)
            nc.vector.tensor_scalar_mul(cenf[:], cenf[:], varf[:])
            nc.vector.tensor_mul(outf[:], cenf[:], g4_sb[:])
            nc.vector.tensor_add(outf[:], outf[:], b4_sb[:])
            nc.sync.dma_start(out[:], outf[:])

        stack.close()
    nc.compile()
    return nc


_NC_CACHE = []


def _get_nc():
    if not _NC_CACHE:
        _NC_CACHE.append(_build())
    return _NC_CACHE[0]


def _shard_inputs(inputs):
    f32 = np.float32
    ae = np.asarray(inputs["adj_matrix_edges_wo"], f32)
    ane = np.asarray(inputs["atm_dir_edge_adj_matrix"], f32)
    a = np.asarray(inputs["adj_matrix"], f32)
    eal = np.asarray(inputs["edge_aligned_node_features"], f32)
    ef = np.asarray(inputs["dir_edge_features"], f32)
    nf = np.asarray(inputs["node_features"], f32)
    shared = {
        "w_init": np.ascontiguousarray(inputs["W_init"], f32),
        "b_init": np.ascontiguousarray(np.asarray(inputs["b_init"], f32).reshape(H, 1)),
        "w_pass": np.ascontiguousarray(inputs["W_pass"], f32),
        "w_gin": np.ascontiguousarray(inputs["W_gin"], f32),
        "b_gin": np.ascontiguousarray(inputs["b_gin"], f32),
        "eps_t": np.ascontiguousarray(np.asarray(inputs["eps"], f32).reshape(1, G)),
    }
    for k in ("g1", "b1", "g2", "b2"):
        shared[k + "_t"] = np.ascontiguousarray(np.asarray(inputs[k], f32).reshape(1, H))
    for k in ("g3", "b3", "g4", "b4"):
        shared[k + "_t"] = np.ascontiguousarray(np.asarray(inputs[k], f32).reshape(1, G))
    in_maps = []
    for c in range(C):
        m = dict(shared)
        m["ae_s"] = np.ascontiguousarray(ae[c * ES:(c + 1) * ES])
        m["ane_s"] = np.ascontiguousarray(ane[c * NS:(c + 1) * NS])
        m["a_s"] = np.ascontiguousarray(a[c * NS:(c + 1) * NS])
        m["eal_s"] = np.ascontiguousarray(eal[c * ES:(c + 1) * ES])
        m["ef_s"] = np.ascontiguousarray(ef[c * ES:(c + 1) * ES])
        m["nf_s"] = np.ascontiguousarray(nf[c * NS:(c + 1) * NS])
        in_maps.append(m)
    return in_maps


def run(inputs, **spmd_kwargs):
    """Run on hardware; returns (output, BassKernelResults)."""
    nc = _get_nc()
    in_maps = _shard_inputs(inputs)
    res = run_bass_kernel_spmd(nc, in_maps, core_ids=list(range(C)), **spmd_kwargs)
    return res.results[0]["out"], res


def kernel(**inputs) -> np.ndarray:
    out, _ = run(inputs)
    return np.ascontiguousarray(out, dtype=np.float32)
